# revision 4
# baseline (speedup 1.0000x reference)
"""CrossModalCenterLoss on 8 NeuronCores (Bass/Tile).

Reference semantics:
    distmat[b, c] = ||x_b||^2 + ||center_c||^2 - 2 <x_b, center_c>
    loss = sum(clip(distmat * onehot(labels), 1e-12, 1e12)) / B

The mask keeps only distmat[b, labels[b]]; every masked-out entry is exactly
0.0 and clip() lifts it to 1e-12.  So:
    loss = mean_b clip(||x_b - centers[labels[b]]||^2, 1e-12, 1e12)
           + (C - 1) * 1e-12
No [B, C] matmul is needed — just a gather + per-row squared distance.

Sharding: data-parallel over batch.  Each of the 8 cores gets 512 rows of
x/labels; centers are replicated.  On-device per core: gather centers[labels]
via indirect DMA, diff = x - c on DVE, Square+row-accumulate on ACT.  The
[512] per-row distances come back per core; host does clip + f64 sum.
"""

import numpy as np

import concourse.bacc as bacc
import concourse.bass as bass
import concourse.mybir as mybir
from concourse.bass_utils import run_bass_kernel_spmd
from concourse.tile import TileContext

B = 4096
D = 512
C = 10000
N_CORES = 8
ROWS = B // N_CORES  # 512 rows per core
P = 128
NT = ROWS // P  # 4 tiles of 128 rows per core

_nc_cache = None

# Stash of the most recent BassKernelResults (exec_time_ns etc.) for test
# harnesses; harmless in production use.
LAST_RESULT = None


def _build_nc():
    # Bacc (not raw Bass): its compile() splits multi-sem waits into event
    # semaphores — TRN2 allows at most one wait per instruction.
    nc = bacc.Bacc("TRN2", target_bir_lowering=False, num_devices=N_CORES)
    f32 = mybir.dt.float32

    x = nc.dram_tensor("x", [ROWS, D], f32, kind="ExternalInput")
    labels = nc.dram_tensor("labels", [NT, P, 1], mybir.dt.int32, kind="ExternalInput")
    centers = nc.dram_tensor("centers", [C, D], f32, kind="ExternalInput")
    out = nc.dram_tensor("out", [P, NT], f32, kind="ExternalOutput")

    with TileContext(nc) as tc:
        with (
            tc.tile_pool(name="io", bufs=NT) as io_pool,
            tc.tile_pool(name="acc", bufs=1) as acc_pool,
        ):
            d_col = acc_pool.tile([P, NT], f32)
            for t in range(NT):
                x_tile = io_pool.tile([P, D], f32, tag="x")
                nc.sync.dma_start(out=x_tile[:], in_=x[t * P : (t + 1) * P, :])

                idx_tile = io_pool.tile([P, 1], mybir.dt.int32, tag="idx")
                nc.sync.dma_start(out=idx_tile[:], in_=labels[t])

                c_tile = io_pool.tile([P, D], f32, tag="c")
                nc.gpsimd.indirect_dma_start(
                    out=c_tile[:],
                    out_offset=None,
                    in_=centers[:],
                    in_offset=bass.IndirectOffsetOnAxis(ap=idx_tile[:, :1], axis=0),
                )

                diff = io_pool.tile([P, D], f32, tag="diff")
                nc.vector.tensor_tensor(
                    out=diff[:],
                    in0=x_tile[:],
                    in1=c_tile[:],
                    op=mybir.AluOpType.subtract,
                )
                sq = io_pool.tile([P, D], f32, tag="sq")
                nc.scalar.activation(
                    out=sq[:],
                    in_=diff[:],
                    func=mybir.ActivationFunctionType.Square,
                    accum_out=d_col[:, t : t + 1],
                )
            nc.sync.dma_start(out=out[:], in_=d_col[:])
    nc.compile()
    return nc


def kernel(x, labels, centers):
    global _nc_cache, LAST_RESULT
    if _nc_cache is None:
        _nc_cache = _build_nc()
    nc = _nc_cache

    xs = np.ascontiguousarray(np.asarray(x, dtype=np.float32))
    cen = np.ascontiguousarray(np.asarray(centers, dtype=np.float32))
    lab = np.ascontiguousarray(
        np.asarray(labels).astype(np.int32).reshape(N_CORES, NT, P, 1)
    )

    in_maps = [
        {
            "x": xs[i * ROWS : (i + 1) * ROWS],
            "labels": lab[i],
            "centers": cen,
        }
        for i in range(N_CORES)
    ]
    res = run_bass_kernel_spmd(nc, in_maps, core_ids=list(range(N_CORES)))
    LAST_RESULT = res

    # out[p, t] holds d for row t*128 + p of that core's shard
    d = np.concatenate([r["out"].T.reshape(-1) for r in res.results])
    d = np.clip(d.astype(np.float64), 1e-12, 1e12)
    loss = d.sum() / B + (C - 1) * 1e-12
    return np.asarray(loss, dtype=np.float32)


# revision 5
# speedup vs baseline: 1.1321x; 1.1321x over previous
"""CrossModalCenterLoss on 8 NeuronCores (Bass/Tile).

Reference semantics:
    distmat[b, c] = ||x_b||^2 + ||center_c||^2 - 2 <x_b, center_c>
    loss = sum(clip(distmat * onehot(labels), 1e-12, 1e12)) / B

The mask keeps only distmat[b, labels[b]]; every masked-out entry is exactly
0.0 and clip() lifts it to 1e-12.  So:
    loss = mean_b clip(||x_b - centers[labels[b]]||^2, 1e-12, 1e12)
           + (C - 1) * 1e-12
No [B, C] matmul is needed — just a gather + per-row squared distance.

Sharding: data-parallel over batch.  Each of the 8 cores gets 512 rows of
x/labels; centers are replicated.  Per core (Tile framework):
  - one DMA for all 512 labels (int32, [128, 4]; [p, t] = label of row
    t*128 + p)
  - 4 indirect-DMA gathers (128 rows each) of centers[labels] -> SBUF
  - x loaded as 4x [128, 512] chunks of a host-pre-permuted [128, 2048]
    layout (partition p, block t = row t*128 + p)
  - per tile: DVE subtract, ACT Square with fused row-accumulate
  - one [128, 4] DMA out with the per-row squared distances
Host applies clip, sums in f64, divides by B, and adds (C-1)*1e-12.

Per the TRN2 cost model this sits at the structural floor: ~5.9 us of
serialized DMA data (2 MB/core at ~360 GB/s) plus fixed issue/semaphore/
drain overheads; compute (DVE/ACT) is fully hidden.
"""

import numpy as np

import concourse.bacc as bacc
import concourse.bass as bass
import concourse.mybir as mybir
from concourse.bass_utils import run_bass_kernel_spmd
from concourse.tile import TileContext

B = 4096
D = 512
C = 10000
N_CORES = 8
ROWS = B // N_CORES  # 512 rows per core
P = 128
NT = ROWS // P  # 4 tiles of 128 rows per core

_nc_cache = None

# Stash of the most recent BassKernelResults (exec_time_ns etc.) for test
# harnesses; harmless in production use.
LAST_RESULT = None


def _build_nc():
    # Bacc (not raw Bass): its compile() splits multi-sem waits into event
    # semaphores — TRN2 allows at most one wait per instruction.
    nc = bacc.Bacc("TRN2", target_bir_lowering=False, num_devices=N_CORES)
    f32 = mybir.dt.float32

    # x layout: [128, NT*D]; partition p, column block t = batch row t*128+p
    x = nc.dram_tensor("x", [P, NT * D], f32, kind="ExternalInput")
    labels = nc.dram_tensor("labels", [P, NT], mybir.dt.int32, kind="ExternalInput")
    centers = nc.dram_tensor("centers", [C, D], f32, kind="ExternalInput")
    out = nc.dram_tensor("out", [P, NT], f32, kind="ExternalOutput")

    with TileContext(nc) as tc:
        with tc.tile_pool(name="acc", bufs=1) as acc_pool:
            d_col = acc_pool.tile([P, NT], f32)

            idx_tile = acc_pool.tile([P, NT], mybir.dt.int32, tag="idx")
            nc.sync.dma_start(out=idx_tile[:], in_=labels[:])

            c_big = acc_pool.tile([P, NT * D], f32, tag="c")
            for t in range(NT):
                nc.gpsimd.indirect_dma_start(
                    out=c_big[:, t * D : (t + 1) * D],
                    out_offset=None,
                    in_=centers[:],
                    in_offset=bass.IndirectOffsetOnAxis(
                        ap=idx_tile[:, t : t + 1], axis=0
                    ),
                )

            x_big = acc_pool.tile([P, NT * D], f32, tag="x")
            for t in range(NT):
                nc.sync.dma_start(
                    out=x_big[:, t * D : (t + 1) * D], in_=x[:, t * D : (t + 1) * D]
                )

            diff = acc_pool.tile([P, NT * D], f32, tag="diff")
            sq = acc_pool.tile([P, NT * D], f32, tag="sq")
            for t in range(NT):
                sl = slice(t * D, (t + 1) * D)
                nc.vector.tensor_tensor(
                    out=diff[:, sl],
                    in0=x_big[:, sl],
                    in1=c_big[:, sl],
                    op=mybir.AluOpType.subtract,
                )
                nc.scalar.activation(
                    out=sq[:, sl],
                    in_=diff[:, sl],
                    func=mybir.ActivationFunctionType.Square,
                    accum_out=d_col[:, t : t + 1],
                )
            nc.sync.dma_start(out=out[:], in_=d_col[:])
    nc.compile()
    return nc


def kernel(x, labels, centers):
    global _nc_cache, LAST_RESULT
    if _nc_cache is None:
        _nc_cache = _build_nc()
    nc = _nc_cache

    x = np.asarray(x, dtype=np.float32).reshape(B, D)
    labels = np.asarray(labels).reshape(B)
    cen = np.ascontiguousarray(np.asarray(centers, dtype=np.float32))

    # per-core layouts (see _build_nc docstring)
    xs = np.ascontiguousarray(
        x.reshape(N_CORES, NT, P, D).transpose(0, 2, 1, 3).reshape(N_CORES, P, NT * D)
    )
    lab = np.ascontiguousarray(
        labels.astype(np.int32).reshape(N_CORES, NT, P).transpose(0, 2, 1)
    )

    in_maps = [
        {"x": xs[i], "labels": lab[i], "centers": cen} for i in range(N_CORES)
    ]
    res = run_bass_kernel_spmd(nc, in_maps, core_ids=list(range(N_CORES)))
    LAST_RESULT = res

    # out[p, t] holds d for row t*128 + p of that core's shard
    d = np.concatenate([r["out"].T.reshape(-1) for r in res.results])
    d = np.clip(d.astype(np.float64), 1e-12, 1e12)
    loss = d.sum() / B + (C - 1) * 1e-12
    return np.asarray(loss, dtype=np.float32)
